# revision 1
# baseline (speedup 1.0000x reference)
"""LoRA attention kernel for 8 trn2 NeuronCores, tensor-parallel over heads.

Sharding: core s owns heads 2s, 2s+1 (a 128-row slice of the HD=1024 dim).
Each core computes q/k/v projections (base + LoRA fused), attention for its
4 (batch, head) pairs, and a partial output projection; the host sums the 8
partials and adds b_out.

Layouts (per core, on-chip):
  xT   [C=1024, B*N=4096]   activations transposed (contraction dim C on
                            partitions, 8 chunks of 128)
  qT/kT/vT [128, 4096]      2 heads x 64 dims on partitions
  attention runs in S^T layout: S^T[k, q] = K^T.T @ Q^T per 128-key chunk,
  exp via ScalarE (mask folded in as a per-partition additive bias), then
  O^T accumulated with lhsT = [V | ones] so the softmax denominator falls
  out of the same matmuls as PSUM row 64.
"""

import numpy as np

import concourse.bass as bass
import concourse.tile as tile
from concourse import bacc, mybir
from concourse.bass_utils import run_bass_kernel_spmd

H, D, R, C, B, N = 16, 64, 10, 1024, 2, 2048
BN = B * N
SCALING = 1.0 / R
ATT_SCALE = float(D) ** -0.5
NCORES = 8
F32 = mybir.dt.float32
F32R = mybir.dt.float32r
NCH = BN // 512  # 8 n-chunks of 512
CCH = C // 128  # 8 contraction chunks
KCH = N // 128  # 16 key chunks per (b,h)
QCH = N // 512  # 4 query chunks per (b,h)


def build_nc(dbg=False):
    nc = bacc.Bacc("TRN2", target_bir_lowering=False, debug=False,
                   num_devices=NCORES)
    if dbg:
        dbg_q = nc.dram_tensor("dbg_q", [128, BN], F32, kind="ExternalOutput")
        dbg_k = nc.dram_tensor("dbg_k", [128, BN], F32, kind="ExternalOutput")
        dbg_v = nc.dram_tensor("dbg_v", [128, BN], F32, kind="ExternalOutput")
        dbg_ao = nc.dram_tensor("dbg_ao", [128, BN], F32, kind="ExternalOutput")
    xT = nc.dram_tensor("xT", [C, BN], F32R, kind="ExternalInput")
    wqT = nc.dram_tensor("wqT", [C, 128], F32R, kind="ExternalInput")
    wkT = nc.dram_tensor("wkT", [C, 128], F32R, kind="ExternalInput")
    wvT = nc.dram_tensor("wvT", [C, 128], F32R, kind="ExternalInput")
    aT = nc.dram_tensor("aT", [C, 64], F32R, kind="ExternalInput")
    bB = nc.dram_tensor("bB", [42, 256], F32R, kind="ExternalInput")
    bq = nc.dram_tensor("bq", [128, 1], F32, kind="ExternalInput")
    bv = nc.dram_tensor("bv", [128, 1], F32, kind="ExternalInput")
    woT = nc.dram_tensor("woT", [CCH, 128, 128], F32R, kind="ExternalInput")
    idn = nc.dram_tensor("idn", [128, 128], F32R, kind="ExternalInput")
    ones = nc.dram_tensor("ones", [128, KCH], F32R, kind="ExternalInput")
    mb = nc.dram_tensor("mb", [128, B * KCH], F32, kind="ExternalInput")
    yT = nc.dram_tensor("yT", [CCH, 128, BN], F32, kind="ExternalOutput")

    with tile.TileContext(nc) as tc:
        with (
            tc.tile_pool(name="wts", bufs=1) as wts,
            tc.tile_pool(name="acts", bufs=1) as acts,
            tc.tile_pool(name="xin", bufs=3) as xin,
            tc.tile_pool(name="zt", bufs=2) as ztp,
            tc.tile_pool(name="pt", bufs=6) as ptp,
            tc.tile_pool(name="vsb", bufs=2) as vsbp,
            tc.tile_pool(name="rec", bufs=2) as recp,
            tc.tile_pool(name="rbc", bufs=2) as rbcp,
            tc.tile_pool(name="yout", bufs=4) as youtp,
            tc.tile_pool(name="ps_s", bufs=2, space="PSUM") as ps_s,
            tc.tile_pool(name="ps_s2", bufs=2, space="PSUM") as ps_s2,
            tc.tile_pool(name="ps_o", bufs=2, space="PSUM") as ps_o,
        ):
            # --- resident weights ---
            wq_s = wts.tile([128, CCH, 128], F32R)
            nc.sync.dma_start(wq_s[:], wqT.ap().rearrange("(i p) m -> p i m", p=128))
            wk_s = wts.tile([128, CCH, 128], F32R)
            nc.sync.dma_start(wk_s[:], wkT.ap().rearrange("(i p) m -> p i m", p=128))
            wv_s = wts.tile([128, CCH, 128], F32R)
            nc.sync.dma_start(wv_s[:], wvT.ap().rearrange("(i p) m -> p i m", p=128))
            a_s = wts.tile([128, CCH, 64], F32R)
            nc.sync.dma_start(a_s[:], aT.ap().rearrange("(i p) m -> p i m", p=128))
            bB_s = wts.tile([42, 256], F32R)
            nc.sync.dma_start(bB_s[:], bB.ap())
            bq_s = wts.tile([128, 1], F32)
            nc.sync.dma_start(bq_s[:], bq.ap())
            bv_s = wts.tile([128, 1], F32)
            nc.sync.dma_start(bv_s[:], bv.ap())
            wo_s = wts.tile([128, CCH, 128], F32R)
            nc.sync.dma_start(wo_s[:], woT.ap().rearrange("i p m -> p i m"))
            mb_s = wts.tile([128, B * KCH], F32)
            nc.sync.dma_start(mb_s[:], mb.ap())
            ident = wts.tile([128, 128], F32R)
            nc.sync.dma_start(ident[:], idn.ap())
            ones_s = wts.tile([128, KCH], F32R)
            nc.sync.dma_start(ones_s[:], ones.ap())

            # --- persistent activations ---
            qT = acts.tile([128, BN], F32R)
            kT = acts.tile([128, BN], F32R)
            vT = acts.tile([128, BN], F32R)
            aoT = acts.tile([128, BN], F32R)

            xT_r = xT.ap().rearrange("(i p) n -> p i n", p=128)

            # ---------- phase 1: projections ----------
            for nch in range(NCH):
                nsl = bass.ts(nch, 512)
                x_t = xin.tile([128, CCH, 512], F32R)
                nc.sync.dma_start(x_t[:], xT_r[:, :, nsl])

                z_ps = ps_o.tile([64, 512], F32, tag="o")
                for i in range(CCH):
                    nc.tensor.matmul(z_ps[:], (a_s[:, i, :]), (x_t[:, i, :]),
                                     start=(i == 0), stop=(i == CCH - 1))
                z_t = ztp.tile([64, 512], F32R)
                nc.vector.tensor_copy(z_t[:], z_ps[:])

                q_ps = ps_s.tile([128, 512], F32, tag="s")
                for i in range(CCH):
                    nc.tensor.matmul(q_ps[:], (wq_s[:, i, :]), (x_t[:, i, :]),
                                     start=(i == 0), stop=False)
                nc.tensor.matmul(q_ps[:], (bB_s[0:R, 0:128]), (z_t[0:R, :]),
                                 start=False, stop=True)
                nc.scalar.activation(qT[:, nsl], q_ps[:],
                                     mybir.ActivationFunctionType.Identity,
                                     bias=bq_s[:])

                k_ps = ps_s.tile([128, 512], F32, tag="s")
                for i in range(CCH):
                    nc.tensor.matmul(k_ps[:], (wk_s[:, i, :]), (x_t[:, i, :]),
                                     start=(i == 0), stop=(i == CCH - 1))
                nc.vector.tensor_copy(kT[:, nsl], k_ps[:])

                v_ps = ps_s.tile([128, 512], F32, tag="s")
                for i in range(CCH):
                    nc.tensor.matmul(v_ps[:], (wv_s[:, i, :]), (x_t[:, i, :]),
                                     start=(i == 0), stop=False)
                nc.tensor.matmul(v_ps[:], (bB_s[32:32 + R, 128:256]),
                                 (z_t[32:32 + R, :]), start=False, stop=True)
                nc.scalar.activation(vT[:, nsl], v_ps[:],
                                     mybir.ActivationFunctionType.Identity,
                                     bias=bv_s[:])

            # ---------- phase 2: attention ----------
            for b in range(B):
                for hh in range(2):
                    hsl = bass.ds(hh * 64, 64)
                    kb = b * N
                    v_sb = vsbp.tile([128, KCH, 65], F32R)
                    nc.vector.tensor_copy(v_sb[:, :, 64:65], ones_s[:])
                    for g in range(2):
                        vt_ps = ps_s.tile([128, 8, 64], F32R, tag="s")
                        for j in range(8):
                            kc = g * 8 + j
                            nc.tensor.transpose(
                                vt_ps[:, j, :],
                                vT[hsl, bass.ds(kb + kc * 128, 128)],
                                ident[hsl, hsl])
                        nc.vector.tensor_copy(
                            v_sb[:, g * 8:(g + 1) * 8, 0:64], vt_ps[:])

                    for qc in range(QCH):
                        qsl = bass.ds(kb + qc * 512, 512)
                        q_ap = qT[hsl, qsl]
                        o_ps = ps_o.tile([65, 512], F32, tag="o")
                        for g in range(KCH // 2):
                            s_ps = ps_s2.tile([128, 2, 512], F32, tag="s2")
                            for j in range(2):
                                kc = g * 2 + j
                                nc.tensor.matmul(
                                    s_ps[:, j, :],
                                    (kT[hsl, bass.ds(kb + kc * 128, 128)]),
                                    (q_ap), start=True, stop=True)
                            p_sb = ptp.tile([128, 2, 512], F32R)
                            nc.scalar.activation(
                                p_sb[:], s_ps[:],
                                mybir.ActivationFunctionType.Exp,
                                bias=mb_s[:, bass.ds(b * KCH + g * 2, 1)],
                                scale=ATT_SCALE)
                            for j in range(2):
                                kc = g * 2 + j
                                nc.tensor.matmul(o_ps[:], (v_sb[:, kc, :]),
                                                 (p_sb[:, j, :]),
                                                 start=(kc == 0),
                                                 stop=(kc == KCH - 1))
                        rec = recp.tile([1, 512], F32)
                        nc.vector.reciprocal(rec[:], o_ps[64:65, :])
                        rbc = rbcp.tile([64, 512], F32)
                        nc.gpsimd.partition_broadcast(rbc[:], rec[:])
                        nc.vector.tensor_mul(aoT[hsl, qsl], o_ps[0:64, :], rbc[:])

            if dbg:
                nc.sync.dma_start(dbg_q.ap(), qT[:].bitcast(F32))
                nc.sync.dma_start(dbg_k.ap(), kT[:].bitcast(F32))
                nc.sync.dma_start(dbg_v.ap(), vT[:].bitcast(F32))
                nc.sync.dma_start(dbg_ao.ap(), aoT[:].bitcast(F32))

            # ---------- phase 3: output projection ----------
            for nch in range(NCH):
                nsl = bass.ts(nch, 512)
                for ci in range(CCH):
                    y_ps = ps_s.tile([128, 512], F32, tag="s")
                    nc.tensor.matmul(y_ps[:], (wo_s[:, ci, :]), (aoT[:, nsl]),
                                     start=True, stop=True)
                    y_sb = youtp.tile([128, 512], F32)
                    if ci % 2 == 0:
                        nc.scalar.copy(y_sb[:], y_ps[:])
                    else:
                        nc.vector.tensor_copy(y_sb[:], y_ps[:])
                    nc.sync.dma_start(yT.ap()[ci, :, nsl], y_sb[:])
    nc.compile()
    return nc


_NC = None


def _get_nc():
    global _NC
    if _NC is None:
        _NC = build_nc()
    return _NC


def _bB(Bq_sl, Bv_sl):
    out = np.zeros((42, 256), np.float32)
    out[0:R, 0:128] = (Bq_sl * SCALING).T
    out[32:32 + R, 128:256] = (Bv_sl * SCALING).T
    return out


def _prep_in_maps(inputs):
    x = np.asarray(inputs["x"], np.float32)
    mask = np.asarray(inputs["mask"])
    W_qkv = np.asarray(inputs["W_qkv"], np.float32)
    Wq_base = np.asarray(inputs["Wq_base"], np.float32)
    bq = np.asarray(inputs["bq"], np.float32)
    Aq = np.asarray(inputs["Aq"], np.float32)
    Bq = np.asarray(inputs["Bq"], np.float32)
    Wv_base = np.asarray(inputs["Wv_base"], np.float32)
    bv = np.asarray(inputs["bv"], np.float32)
    Av = np.asarray(inputs["Av"], np.float32)
    Bv = np.asarray(inputs["Bv"], np.float32)
    W_out = np.asarray(inputs["W_out"], np.float32)

    xT = np.ascontiguousarray(x.reshape(BN, C).T)
    Wq_eff = W_qkv[0:H * D] + Wq_base
    Wk = W_qkv[H * D:2 * H * D]
    Wv_eff = W_qkv[2 * H * D:3 * H * D] + Wv_base
    aT = np.zeros((C, 64), np.float32)
    aT[:, 0:R] = Aq.T
    aT[:, 32:32 + R] = Av.T
    mbias = np.where(mask.reshape(BN), 0.0, -1e5).astype(np.float32)
    mb = np.ascontiguousarray(mbias.reshape(B * KCH, 128).T)

    in_maps = []
    for s in range(NCORES):
        sl = slice(s * 128, (s + 1) * 128)
        in_maps.append({
            "xT": xT,
            "wqT": np.ascontiguousarray(Wq_eff[sl].T),
            "wkT": np.ascontiguousarray(Wk[sl].T),
            "wvT": np.ascontiguousarray(Wv_eff[sl].T),
            "aT": aT,
            "bB": _bB(Bq[sl], Bv[sl]),
            "bq": np.ascontiguousarray(bq[sl, None]),
            "bv": np.ascontiguousarray(bv[sl, None]),
            "woT": np.ascontiguousarray(
                W_out[:, sl].reshape(CCH, 128, 128).transpose(0, 2, 1)),
            "mb": mb,
            "idn": np.eye(128, dtype=np.float32),
            "ones": np.ones((128, KCH), np.float32),
        })
    return in_maps


def _assemble(results, b_out):
    acc = np.zeros((C, BN), np.float64)
    for r in results:
        acc += r["yT"].reshape(C, BN)
    out = acc.T.astype(np.float32) + np.asarray(b_out, np.float32)[None, :]
    return np.ascontiguousarray(out.reshape(B, N, C))


def kernel(**inputs):
    nc = _get_nc()
    in_maps = _prep_in_maps(inputs)
    res = run_bass_kernel_spmd(nc, in_maps, core_ids=list(range(NCORES)))
    return _assemble(res.results, inputs["b_out"])


def run_traced(inputs):
    """test harness hook: returns (output, exec_time_ns)."""
    nc = _get_nc()
    in_maps = _prep_in_maps(inputs)
    res = run_bass_kernel_spmd(nc, in_maps, core_ids=list(range(NCORES)),
                               trace=True)
    return _assemble(res.results, inputs["b_out"]), res.exec_time_ns



# revision 4
# speedup vs baseline: 18.2692x; 18.2692x over previous
"""LoRA attention kernel for 8 trn2 NeuronCores, tensor-parallel over heads.

Compute sharding: core s owns heads 2s, 2s+1 (a 128-row slice of the HD=1024
dim). Each core computes q/k/v projections (base + LoRA fused), attention for
its 4 (batch, head) pairs, and a partial output projection.

I/O sharding (the axon host<->device tunnel is the bottleneck, ~50-100MB/s):
  - x is uploaded token-sharded: core s gets tokens [512s, 512(s+1)) as
    xTl [C, 512] bf16 (~1MB/core), then an on-device AllGather replicates
    the full xT [8, C, 512] to every core.
  - the 8 partial output projections are summed on-device with a
    ReduceScatter, so core s downloads only y[:, 512s:512(s+1)] + b_out as
    yTo [CCH, 128, 512] bf16 (~1MB/core).
  - weights are kept device-resident across calls (re-uploaded only when
    their values change); donated output buffers are created on-device.

On-chip layouts (per core):
  xT   [C=1024, 4096]   activations transposed (contraction dim C on
                        partitions, 8 chunks of 128)
  qT/kT/vT [128, 4096]  2 heads x 64 dims on partitions, bf16
  attention runs in S^T layout: S^T[k, q] = K^T.T @ Q^T per 128-key chunk,
  exp via ScalarE (mask folded in as a per-partition additive bias), then
  O^T accumulated with lhsT = [V | ones] so the softmax denominator falls
  out of the same matmuls as PSUM row 64.
"""

import os
import numpy as np
import ml_dtypes

os.environ.setdefault("JAX_PLATFORMS", "axon")

import jax
import concourse.bass as bass
import concourse.tile as tile
from concourse import bacc, mybir

H, D, R, C, B, N = 16, 64, 10, 1024, 2, 2048
BN = B * N
SCALING = 1.0 / R
ATT_SCALE = float(D) ** -0.5
NCORES = 8
F32 = mybir.dt.float32
BF16 = mybir.dt.bfloat16
NPBF16 = ml_dtypes.bfloat16
NCH = BN // 512  # 8 n-chunks of 512 (chunk s = core s's token block)
CCH = C // 128  # 8 contraction chunks
KCH = N // 128  # 16 key chunks per (b,h)
QCH = N // 512  # 4 query chunks per (b,h)
TOK = BN // NCORES  # 512 tokens per core


def build_nc():
    nc = bacc.Bacc("TRN2", target_bir_lowering=False, debug=False,
                   num_devices=NCORES)
    xTl = nc.dram_tensor("xTl", [C, TOK], BF16, kind="ExternalInput")
    wqT = nc.dram_tensor("wqT", [C, 128], BF16, kind="ExternalInput")
    wkT = nc.dram_tensor("wkT", [C, 128], BF16, kind="ExternalInput")
    wvT = nc.dram_tensor("wvT", [C, 128], BF16, kind="ExternalInput")
    aT = nc.dram_tensor("aT", [C, 64], BF16, kind="ExternalInput")
    bB = nc.dram_tensor("bB", [42, 256], BF16, kind="ExternalInput")
    bq = nc.dram_tensor("bq", [128, 1], F32, kind="ExternalInput")
    bv = nc.dram_tensor("bv", [128, 1], F32, kind="ExternalInput")
    bo = nc.dram_tensor("bo", [CCH, 128, 1], F32, kind="ExternalInput")
    woT = nc.dram_tensor("woT", [CCH, 128, 128], BF16, kind="ExternalInput")
    idn = nc.dram_tensor("idn", [128, 128], BF16, kind="ExternalInput")
    ones = nc.dram_tensor("ones", [128, KCH], BF16, kind="ExternalInput")
    mb = nc.dram_tensor("mb", [128, B * KCH], F32, kind="ExternalInput")
    yTo = nc.dram_tensor("yTo", [CCH, 128, TOK], BF16, kind="ExternalOutput")

    # collective bounce buffers
    ag_in = nc.dram_tensor("ag_in", [C, TOK], BF16, kind="Internal")
    ag_out = nc.dram_tensor("ag_out", [NCORES, C, TOK], BF16, kind="Internal",
                            addr_space="Shared")
    rs_in = nc.dram_tensor("rs_in", [NCH, CCH, 128, TOK], F32, kind="Internal")
    rs_out = nc.dram_tensor("rs_out", [CCH, 128, TOK], F32, kind="Internal")

    groups = [list(range(NCORES))]

    with tile.TileContext(nc) as tc:
        with (
            tc.tile_pool(name="wts", bufs=1) as wts,
            tc.tile_pool(name="acts", bufs=1) as acts,
            tc.tile_pool(name="xin", bufs=3) as xin,
            tc.tile_pool(name="zt", bufs=2) as ztp,
            tc.tile_pool(name="pt", bufs=6) as ptp,
            tc.tile_pool(name="vsb", bufs=2) as vsbp,
            tc.tile_pool(name="rec", bufs=2) as recp,
            tc.tile_pool(name="rbc", bufs=2) as rbcp,
            tc.tile_pool(name="yout", bufs=4) as youtp,
            tc.tile_pool(name="yc", bufs=2) as ycp,
            tc.tile_pool(name="yo", bufs=2) as yop,
            tc.tile_pool(name="ps_s", bufs=2, space="PSUM") as ps_s,
            tc.tile_pool(name="ps_s2", bufs=2, space="PSUM") as ps_s2,
            tc.tile_pool(name="ps_o", bufs=2, space="PSUM") as ps_o,
        ):
            # --- all-gather x across the 8 cores ---
            nc.sync.dma_start(ag_in.ap(), xTl.ap())
            nc.gpsimd.collective_compute(
                "AllGather", mybir.AluOpType.bypass, replica_groups=groups,
                ins=[ag_in.ap()], outs=[ag_out.ap()])

            # --- resident weights ---
            wq_s = wts.tile([128, CCH, 128], BF16)
            nc.sync.dma_start(wq_s[:], wqT.ap().rearrange("(i p) m -> p i m", p=128))
            wk_s = wts.tile([128, CCH, 128], BF16)
            nc.sync.dma_start(wk_s[:], wkT.ap().rearrange("(i p) m -> p i m", p=128))
            wv_s = wts.tile([128, CCH, 128], BF16)
            nc.sync.dma_start(wv_s[:], wvT.ap().rearrange("(i p) m -> p i m", p=128))
            a_s = wts.tile([128, CCH, 64], BF16)
            nc.sync.dma_start(a_s[:], aT.ap().rearrange("(i p) m -> p i m", p=128))
            bB_s = wts.tile([42, 256], BF16)
            nc.sync.dma_start(bB_s[:], bB.ap())
            bq_s = wts.tile([128, 1], F32)
            nc.sync.dma_start(bq_s[:], bq.ap())
            bv_s = wts.tile([128, 1], F32)
            nc.sync.dma_start(bv_s[:], bv.ap())
            bo_s = wts.tile([128, CCH, 1], F32)
            nc.sync.dma_start(bo_s[:], bo.ap().rearrange("i p m -> p i m"))
            wo_s = wts.tile([128, CCH, 128], BF16)
            nc.sync.dma_start(wo_s[:], woT.ap().rearrange("i p m -> p i m"))
            mb_s = wts.tile([128, B * KCH], F32)
            nc.sync.dma_start(mb_s[:], mb.ap())
            ident = wts.tile([128, 128], BF16)
            nc.sync.dma_start(ident[:], idn.ap())
            ones_s = wts.tile([128, KCH], BF16)
            nc.sync.dma_start(ones_s[:], ones.ap())

            # --- persistent activations ---
            qT = acts.tile([128, BN], BF16)
            kT = acts.tile([128, BN], BF16)
            vT = acts.tile([128, BN], BF16)
            aoT = acts.tile([128, BN], BF16)

            xg_r = ag_out.ap().rearrange("r (i p) n -> r p i n", p=128)

            # ---------- phase 1: projections ----------
            for nch in range(NCH):
                nsl = bass.ts(nch, 512)
                x_t = xin.tile([128, CCH, 512], BF16)
                nc.sync.dma_start(x_t[:], xg_r[nch])

                z_ps = ps_o.tile([64, 512], F32, tag="o")
                for i in range(CCH):
                    nc.tensor.matmul(z_ps[:], (a_s[:, i, :]), (x_t[:, i, :]),
                                     start=(i == 0), stop=(i == CCH - 1))
                z_t = ztp.tile([64, 512], BF16)
                nc.vector.tensor_copy(z_t[:], z_ps[:])

                q_ps = ps_s.tile([128, 512], F32, tag="s")
                for i in range(CCH):
                    nc.tensor.matmul(q_ps[:], (wq_s[:, i, :]), (x_t[:, i, :]),
                                     start=(i == 0), stop=False)
                nc.tensor.matmul(q_ps[:], (bB_s[0:R, 0:128]), (z_t[0:R, :]),
                                 start=False, stop=True)
                nc.scalar.activation(qT[:, nsl], q_ps[:],
                                     mybir.ActivationFunctionType.Identity,
                                     bias=bq_s[:])

                k_ps = ps_s.tile([128, 512], F32, tag="s")
                for i in range(CCH):
                    nc.tensor.matmul(k_ps[:], (wk_s[:, i, :]), (x_t[:, i, :]),
                                     start=(i == 0), stop=(i == CCH - 1))
                nc.vector.tensor_copy(kT[:, nsl], k_ps[:])

                v_ps = ps_s.tile([128, 512], F32, tag="s")
                for i in range(CCH):
                    nc.tensor.matmul(v_ps[:], (wv_s[:, i, :]), (x_t[:, i, :]),
                                     start=(i == 0), stop=False)
                nc.tensor.matmul(v_ps[:], (bB_s[32:32 + R, 128:256]),
                                 (z_t[32:32 + R, :]), start=False, stop=True)
                nc.scalar.activation(vT[:, nsl], v_ps[:],
                                     mybir.ActivationFunctionType.Identity,
                                     bias=bv_s[:])

            # ---------- phase 2: attention ----------
            for b in range(B):
                for hh in range(2):
                    hsl = bass.ds(hh * 64, 64)
                    kb = b * N
                    v_sb = vsbp.tile([128, KCH, 65], BF16)
                    nc.vector.tensor_copy(v_sb[:, :, 64:65], ones_s[:])
                    for g in range(2):
                        vt_ps = ps_s.tile([128, 8, 64], BF16, tag="s")
                        for j in range(8):
                            kc = g * 8 + j
                            nc.tensor.transpose(
                                vt_ps[:, j, :],
                                vT[hsl, bass.ds(kb + kc * 128, 128)],
                                ident[hsl, hsl])
                        nc.vector.tensor_copy(
                            v_sb[:, g * 8:(g + 1) * 8, 0:64], vt_ps[:])

                    for qc in range(QCH):
                        qsl = bass.ds(kb + qc * 512, 512)
                        q_ap = qT[hsl, qsl]
                        o_ps = ps_o.tile([65, 512], F32, tag="o")
                        for g in range(KCH // 2):
                            s_ps = ps_s2.tile([128, 2, 512], F32, tag="s2")
                            for j in range(2):
                                kc = g * 2 + j
                                nc.tensor.matmul(
                                    s_ps[:, j, :],
                                    (kT[hsl, bass.ds(kb + kc * 128, 128)]),
                                    (q_ap), start=True, stop=True)
                            p_sb = ptp.tile([128, 2, 512], BF16)
                            nc.scalar.activation(
                                p_sb[:], s_ps[:],
                                mybir.ActivationFunctionType.Exp,
                                bias=mb_s[:, bass.ds(b * KCH + g * 2, 1)],
                                scale=ATT_SCALE)
                            for j in range(2):
                                kc = g * 2 + j
                                nc.tensor.matmul(o_ps[:], (v_sb[:, kc, :]),
                                                 (p_sb[:, j, :]),
                                                 start=(kc == 0),
                                                 stop=(kc == KCH - 1))
                        rec = recp.tile([1, 512], F32)
                        nc.vector.reciprocal(rec[:], o_ps[64:65, :])
                        rbc = rbcp.tile([64, 512], F32)
                        nc.gpsimd.partition_broadcast(rbc[:], rec[:])
                        nc.vector.tensor_mul(aoT[hsl, qsl], o_ps[0:64, :], rbc[:])

            # ---------- phase 3: output projection (partial sums) ----------
            for nch in range(NCH):
                nsl = bass.ts(nch, 512)
                for ci in range(CCH):
                    y_ps = ps_s.tile([128, 512], F32, tag="s")
                    nc.tensor.matmul(y_ps[:], (wo_s[:, ci, :]), (aoT[:, nsl]),
                                     start=True, stop=True)
                    y_sb = youtp.tile([128, 512], F32)
                    if ci % 2 == 0:
                        nc.scalar.copy(y_sb[:], y_ps[:])
                    else:
                        nc.vector.tensor_copy(y_sb[:], y_ps[:])
                    nc.sync.dma_start(rs_in.ap()[nch, ci], y_sb[:])

            # ---------- reduce-scatter partials, add b_out, emit bf16 ----------
            nc.gpsimd.collective_compute(
                "ReduceScatter", mybir.AluOpType.add, replica_groups=groups,
                ins=[rs_in.ap()], outs=[rs_out.ap()])
            for ci in range(CCH):
                yc = ycp.tile([128, TOK], F32)
                nc.sync.dma_start(yc[:], rs_out.ap()[ci])
                yo = yop.tile([128, TOK], BF16)
                nc.scalar.activation(yo[:], yc[:],
                                     mybir.ActivationFunctionType.Identity,
                                     bias=bo_s[:, ci, :])
                nc.sync.dma_start(yTo.ap()[ci], yo[:])
    nc.compile()
    return nc


def _bB(Bq_sl, Bv_sl):
    out = np.zeros((42, 256), np.float32)
    out[0:R, 0:128] = (Bq_sl * SCALING).T
    out[32:32 + R, 128:256] = (Bv_sl * SCALING).T
    return out


_WEIGHT_KEYS = ("mask", "W_qkv", "Wq_base", "bq", "Aq", "Bq", "Wv_base",
                "bv", "Av", "Bv", "W_out", "b_out")


def _prep_weight_maps(inputs):
    """Per-core weight input dicts (everything except x)."""
    mask = np.asarray(inputs["mask"])
    W_qkv = np.asarray(inputs["W_qkv"], np.float32)
    Wq_base = np.asarray(inputs["Wq_base"], np.float32)
    bq = np.asarray(inputs["bq"], np.float32)
    Aq = np.asarray(inputs["Aq"], np.float32)
    Bq = np.asarray(inputs["Bq"], np.float32)
    Wv_base = np.asarray(inputs["Wv_base"], np.float32)
    bv = np.asarray(inputs["bv"], np.float32)
    Av = np.asarray(inputs["Av"], np.float32)
    Bv = np.asarray(inputs["Bv"], np.float32)
    W_out = np.asarray(inputs["W_out"], np.float32)
    b_out = np.asarray(inputs["b_out"], np.float32)

    Wq_eff = W_qkv[0:H * D] + Wq_base
    Wk = W_qkv[H * D:2 * H * D]
    Wv_eff = W_qkv[2 * H * D:3 * H * D] + Wv_base
    aT = np.zeros((C, 64), np.float32)
    aT[:, 0:R] = Aq.T
    aT[:, 32:32 + R] = Av.T
    mbias = np.where(mask.reshape(BN), 0.0, -1e5).astype(np.float32)
    mbv = np.ascontiguousarray(mbias.reshape(B * KCH, 128).T)
    bo = np.ascontiguousarray(b_out.reshape(CCH, 128, 1))

    maps = []
    for s in range(NCORES):
        sl = slice(s * 128, (s + 1) * 128)
        maps.append({
            "wqT": np.ascontiguousarray(Wq_eff[sl].T).astype(NPBF16),
            "wkT": np.ascontiguousarray(Wk[sl].T).astype(NPBF16),
            "wvT": np.ascontiguousarray(Wv_eff[sl].T).astype(NPBF16),
            "aT": aT.astype(NPBF16),
            "bB": _bB(Bq[sl], Bv[sl]).astype(NPBF16),
            "bq": np.ascontiguousarray(bq[sl, None]),
            "bv": np.ascontiguousarray(bv[sl, None]),
            "bo": bo,
            "woT": np.ascontiguousarray(
                W_out[:, sl].reshape(CCH, 128, 128).transpose(0, 2, 1)
            ).astype(NPBF16),
            "mb": mbv,
            "idn": np.eye(128, dtype=NPBF16),
            "ones": np.ones((128, KCH), NPBF16),
        })
    return maps


def _prep_x_global(x):
    """Token-sharded global xT: [NCORES*C, TOK] bf16 (core s = rows s*C..)."""
    xf = np.asarray(x, np.float32).reshape(BN, C)
    xg = xf.reshape(NCORES, TOK, C).transpose(0, 2, 1)  # (8, C, TOK)
    return np.ascontiguousarray(xg).astype(NPBF16).reshape(NCORES * C, TOK)


class _Runtime:
    """Caches the compiled Bass module, the jitted sharded executable, and
    device-resident weights across kernel() calls."""

    def __init__(self):
        import jax.numpy as jnp
        from jax.sharding import Mesh, PartitionSpec, NamedSharding
        try:
            from jax.experimental.shard_map import shard_map
        except ImportError:
            from jax import shard_map
        from concourse.bass2jax import (
            install_neuronx_cc_hook, _bass_exec_p, partition_id_tensor)

        self.nc = build_nc()
        nc = self.nc
        install_neuronx_cc_hook()

        partition_name = (nc.partition_id_tensor.name
                          if nc.partition_id_tensor else None)
        in_names, out_names, out_avals = [], [], []
        for alloc in nc.m.functions[0].allocations:
            if not isinstance(alloc, mybir.MemoryLocationSet):
                continue
            name = alloc.memorylocations[0].name
            if alloc.kind == "ExternalInput":
                if name != partition_name:
                    in_names.append(name)
            elif alloc.kind == "ExternalOutput":
                out_names.append(name)
                out_avals.append(jax.core.ShapedArray(
                    tuple(alloc.tensor_shape), mybir.dt.np(alloc.dtype)))
        self.in_names = in_names
        self.out_names = out_names
        self.out_avals = out_avals
        n_params = len(in_names)
        n_outs = len(out_avals)
        in_names_all = list(in_names) + list(out_names)
        if partition_name is not None:
            in_names_all.append(partition_name)

        def _body(*args):
            operands = list(args)
            if partition_name is not None:
                operands.append(partition_id_tensor())
            outs = _bass_exec_p.bind(
                *operands,
                out_avals=tuple(out_avals),
                in_names=tuple(in_names_all),
                out_names=tuple(out_names),
                lowering_input_output_aliases=(),
                sim_require_finite=True,
                sim_require_nnan=True,
                nc=nc,
            )
            return tuple(outs)

        devices = jax.devices()[:NCORES]
        assert len(devices) == NCORES
        self.mesh = Mesh(np.asarray(devices), ("core",))
        self.sharding = NamedSharding(self.mesh, PartitionSpec("core"))
        in_specs = (PartitionSpec("core"),) * (n_params + n_outs)
        out_specs = (PartitionSpec("core"),) * n_outs
        donate = tuple(range(n_params, n_params + n_outs))
        self.sharded = jax.jit(
            shard_map(_body, mesh=self.mesh, in_specs=in_specs,
                      out_specs=out_specs, check_rep=False),
            donate_argnums=donate,
            keep_unused=True,
        )

        zshapes = [(NCORES * a.shape[0], *a.shape[1:]) for a in out_avals]
        zdtypes = [a.dtype for a in out_avals]
        self._zeros = jax.jit(
            lambda: tuple(jnp.zeros(s, d) for s, d in zip(zshapes, zdtypes)),
            out_shardings=tuple(self.sharding for _ in out_avals),
        )

        self._w_src = None  # raw weight arrays for change detection
        self._w_dev = None  # name -> sharded device array

    def _weights_device(self, inputs):
        cur = [np.asarray(inputs[k]) for k in _WEIGHT_KEYS]
        if self._w_src is not None and all(
                np.array_equal(a, b) for a, b in zip(self._w_src, cur)):
            return self._w_dev
        maps = _prep_weight_maps(inputs)
        dev = {}
        for name in self.in_names:
            if name == "xTl":
                continue
            g = np.concatenate([maps[s][name] for s in range(NCORES)], axis=0)
            dev[name] = jax.device_put(g, self.sharding)
        self._w_src = [a.copy() for a in cur]
        self._w_dev = dev
        return dev

    def run(self, inputs):
        wdev = self._weights_device(inputs)
        xg = _prep_x_global(inputs["x"])
        args = [xg if name == "xTl" else wdev[name] for name in self.in_names]
        zeros = self._zeros()
        out = self.sharded(*args, *zeros)
        og = np.asarray(out[self.out_names.index("yTo")])
        # (NCORES*CCH, 128, TOK) bf16 -> (B, N, C) f32
        of = og.astype(np.float32).reshape(NCORES, C, TOK)
        y = of.transpose(0, 2, 1).reshape(BN, C)
        return np.ascontiguousarray(y.reshape(B, N, C))


_RT = None


def _get_rt():
    global _RT
    if _RT is None:
        _RT = _Runtime()
    return _RT


def kernel(**inputs):
    return _get_rt().run(inputs)
